# revision 27
# baseline (speedup 1.0000x reference)
"""Trainium2 Bass kernel for nn_MultiHeadAttention_63015760167496.

Computation (see reference): qkv = x @ Wqkv; RoPE on q,k; causal softmax
attention per head; out = einsum('bhts,bshd->bhtd', probs, v);
out.reshape(B,T,C) @ Wout  -- NOTE the reshape is a *head-major* flatten of
[B,H,T,D] into [B,T,C], so final-output row r = h*128 + t//16 depends only on
head h.  Sharding: head-parallel over 8 cores (2 heads/core); every core
computes its two heads end-to-end and produces final-output rows
[256*i, 256*i+256).  Host concatenates -- no collectives.

The QKV projection runs as float32r (TF32-like, 1 col/cycle at N>=256).
q/k/v (post-RoPE), attention probabilities, attention outputs and Wout are
bf16: same PE rate with no fp32r N>=256 restriction, 2x DVE throughput for
sbuf-only elementwise ops, half the Wout DMA.  Attention uses the S^T
layout ([s,t]): softmax denominator via a ones-vector matmul (partition
reduction on the PE), normalization via a K=1 broadcast matmul.  No running
max is needed (scores are O(5), fp32 psum).

The TRN2 PE clock ramps (0.65 -> 1.2 -> 2.4 GHz) only under *continuous*
load; any idle gap drops it back.  A bare attention phase is paced by the
scalar-engine Exp and keeps micro-stalling the PE, which then runs its
matmuls at roughly half clock.  So the schedule interleaves phases that
have independent tensor work:
    A: qkv(b=0)
    B: attention(b=0) + qkv(b=1)        (interleaved emission)
    C: attention(b=1) + out-proj(b=0)   (interleaved emission)
    D: out-proj(b=1)
The interleaving is done by a cost-weighted round-robin merge of per-phase
emission generators; the Tile framework's semaphores keep it correct.
"""

import math
import sys

for _p in ("/opt/trn_rl_repo", "/root/.axon_site/_ro/trn_rl_repo"):
    if _p not in sys.path:
        sys.path.insert(0, _p)

import numpy as np
import ml_dtypes

import concourse.bass as bass
import concourse.mybir as mybir
import concourse.tile as tile
from concourse import bacc
from concourse.bass_utils import run_bass_kernel_spmd

B, T, C = 2, 2048, 2048
H = 16            # heads total
D = C // H        # 128 head dim
HALF = D // 2     # 64
P = 128
KO = C // P       # 16 contraction chunks
NCORES = 8
HPC = H // NCORES  # 2 heads per core
TQ = 256          # t-tile for qkv projection
NT = T // TQ
TA = 512          # t-tile for attention
NTA = T // TA
SCPT = TA // P    # 4 s-chunks per attention tile
TC_ = 256         # out-projection column tile
NCP = C // TC_
ROPE_BASE = 10000.0
SCALE = 1.0 / math.sqrt(D)

f32 = mybir.dt.float32
f32r = mybir.dt.float32r
bf16 = mybir.dt.bfloat16


def _merge(*gens):
    """Cost-weighted round-robin: always step the generator with the least
    accumulated emitted-tensor-time.  Generators yield ns estimates."""
    acc = [0.0] * len(gens)
    live = list(range(len(gens)))
    while live:
        i = min(live, key=lambda k: acc[k])
        try:
            acc[i] += next(gens[i])
        except StopIteration:
            live.remove(i)


def _build():
    nc = bacc.Bacc("TRN2", target_bir_lowering=False, debug=False,
                   num_devices=NCORES)

    # host-pre-tiled x^T: xTt[b, ti, p, ko, u] = x[b, ti*TQ+u, ko*128+p]
    xTt = nc.dram_tensor("xTt", [B, NT, P, KO, TQ], bf16, kind="ExternalInput")
    # host-pre-chunked weights: wq/wk[p, hh, ko, d] = W[ko*128+p, hh*128+d]
    wq = nc.dram_tensor("wq", [P, HPC, KO, D], bf16, kind="ExternalInput")
    wk = nc.dram_tensor("wk", [P, HPC, KO, D], bf16, kind="ExternalInput")
    wv = nc.dram_tensor("wv", [P, KO, HPC * D], bf16, kind="ExternalInput")
    # woutp[cpi, p, j, m] = Wout[j*128+p, cpi*TC_+m], bf16
    woutp = nc.dram_tensor("woutp", [NCP, P, KO, TC_], bf16,
                           kind="ExternalInput")
    cs2 = nc.dram_tensor("cs2", [P, T], f32, kind="ExternalInput")  # [cos;cos]
    sn1 = nc.dram_tensor("sn1", [HALF, T], f32, kind="ExternalInput")  # sin
    # tri[s, u] = 1 iff s <= u (valid upper triangle in the S^T layout)
    tri = nc.dram_tensor("tri", [P, P], bf16, kind="ExternalInput")
    y = nc.dram_tensor("y", [B, HPC * D, C], f32, kind="ExternalOutput")

    with tile.TileContext(nc) as tc:
        with tc.tile_pool(name="const", bufs=1) as cp_, \
             tc.tile_pool(name="qkv", bufs=1) as qp, \
             tc.tile_pool(name="ot", bufs=1) as op_, \
             tc.tile_pool(name="wo", bufs=4) as wop, \
             tc.tile_pool(name="small", bufs=3) as sp, \
             tc.tile_pool(name="pt", bufs=6) as ptp, \
             tc.tile_pool(name="psBsc", bufs=2, space="PSUM") as pssc, \
             tc.tile_pool(name="psBo", bufs=1, space="PSUM") as pso, \
             tc.tile_pool(name="psBsum", bufs=1, space="PSUM") as pssum:

            wq_sb = cp_.tile([P, HPC, KO, D], bf16, tag="wq")
            wk_sb = cp_.tile([P, HPC, KO, D], bf16, tag="wk")
            wv_sb = cp_.tile([P, KO, HPC * D], bf16, tag="wv")
            # first q matmuls need only wq[:, 0]; split the DMA so they
            # start sooner.  cs/sn head chunks come first: RoPE(ti=0) gates
            # the psum-accumulator recycling.
            nc.sync.dma_start(wq_sb[:, 0], wq.ap()[:, 0])
            cs_sb = cp_.tile([P, T], f32, tag="cs")
            sn_sb = cp_.tile([HALF, T], f32, tag="sn")
            tri_sb = cp_.tile([P, P], bf16, tag="tri")
            # ones *matrix* stationary for the denominator matmuls: same
            # cycle cost as a ones-vector (cost ~ moving size), but keeps
            # the PE tile config at (128,128) -- a (128,32) sum config
            # between the score/o matmuls breaks ldweights pipelining --
            # and broadcasts the sums to all partitions, which makes the
            # normalization a plain elementwise multiply (no K=1 broadcast
            # matmul needed).
            ones_f32 = cp_.tile([P, P], f32, tag="ones_f32")
            nc.vector.memset(ones_f32[:], 1.0)
            ones_mat = cp_.tile([P, P], bf16, tag="ones_mat")
            nc.vector.tensor_copy(ones_mat[:], ones_f32[:])

            # PE warmup: the clock ramp (0.65->2.4GHz) needs sustained
            # activity; burn cheap dummy matmuls while the first DMAs land.
            warm_sb = cp_.tile([P, TQ], bf16, tag="warm_sb")
            nc.vector.memset(warm_sb[:], 0.0)
            ps_warm = pssc.tile([P, TA], f32, tag="sc", name="warm")
            for wi in range(24):
                nc.tensor.matmul(ps_warm[:, 0:TQ], ones_mat[:], warm_sb[:],
                                 start=True, stop=True)

            # persistent attention outputs O^T per (b, local head): [d, t]
            oT = [[op_.tile([P, T], bf16, tag=f"oT{b}{hh}", name=f"oT{b}{hh}")
                   for hh in range(HPC)] for b in range(B)]
            qT = [[qp.tile([P, T], bf16, tag=f"qT{b}{hh}", name=f"qT{b}{hh}")
                   for hh in range(HPC)] for b in range(B)]
            kT = [[qp.tile([P, T], bf16, tag=f"kT{b}{hh}", name=f"kT{b}{hh}")
                   for hh in range(HPC)] for b in range(B)]
            vt = [[qp.tile([P, T // P, D], bf16, tag=f"v{b}{hh}",
                           name=f"v{b}{hh}")
                   for hh in range(HPC)] for b in range(B)]

            def gen_qkv(b, xp, psa, rp, state):
                for ti in range(NT):
                    sl = slice(ti * TQ, (ti + 1) * TQ)
                    first_tile = (b == 0 and ti == 0)
                    if state.get("next") is not None:
                        xt = state.pop("next")
                    else:
                        xt = xp.tile([P, KO, TQ], bf16, tag="xt",
                                     name=f"xt{b}_{ti}")
                        if not first_tile:
                            nc.sync.dma_start(xt[:], xTt.ap()[b, ti])
                    cs = cs_sb[:, sl]
                    sn = sn_sb[:, sl]  # [64, TQ] base partition 0

                    def qkmm(w_sb, hh, split_dma=False, csn_after=None):
                        ps = psa.tile([P, TQ], f32, tag="acc",
                                      name=f"acc{b}_{ti}_{hh}")
                        for ko in range(KO):
                            if split_dma and ko % 4 == 0:
                                kos = slice(ko, ko + 4)
                                nc.sync.dma_start(xt[:, kos],
                                                  xTt.ap()[b, ti, :, kos])
                                if ko == 4 and csn_after:
                                    nc.sync.dma_start(cs_sb[:, 0:TQ],
                                                      cs2.ap()[:, 0:TQ])
                                    nc.sync.dma_start(sn_sb[:, 0:TQ],
                                                      sn1.ap()[:, 0:TQ])
                            nc.tensor.matmul(ps[:], w_sb[:, hh, ko, :],
                                             xt[:, ko, :],
                                             start=(ko == 0),
                                             stop=(ko == KO - 1))
                        return ps

                    def rope(ps, dst):
                        # tcos = ps * [cos;cos] (one full mult); tsw
                        # pre-swaps halves: tsw[0:64]=q2*sin,
                        # tsw[64:128]=q1*sin so the gpsimd add/sub reads
                        # align on base partitions.
                        tcos = rp.tile([P, TQ], f32, tag="tcos")
                        tsw = rp.tile([P, TQ], f32, tag="tsw")
                        nc.vector.tensor_mul(tcos[:], ps[:], cs)
                        nc.vector.tensor_mul(tsw[0:HALF, :],
                                             ps[HALF:P, :], sn)
                        nc.vector.tensor_mul(tsw[HALF:P, :],
                                             ps[0:HALF, :], sn)
                        nc.gpsimd.tensor_sub(dst[0:HALF, sl],
                                             tcos[0:HALF, :],
                                             tsw[0:HALF, :])
                        nc.gpsimd.tensor_add(dst[HALF:P, sl],
                                             tcos[HALF:P, :],
                                             tsw[HALF:P, :])

                    if first_tile:
                        # q accums first (need only wq + xt0, DMA'd in ko
                        # quarters); stagger the remaining const DMAs
                        # behind them.
                        ps0 = qkmm(wq_sb, 0, split_dma=True,
                                   csn_after=1)
                        nc.sync.dma_start(wq_sb[:, 1], wq.ap()[:, 1])
                        nc.sync.dma_start(wk_sb[:], wk.ap())
                        yield 1700.0
                        ps1 = qkmm(wq_sb, 1)
                        nc.sync.dma_start(wv_sb[:], wv.ap())
                        rope(ps0, qT[b][0])
                        yield 1700.0
                        psk = qkmm(wk_sb, 0)
                        rope(ps1, qT[b][1])
                        rope(psk, kT[b][0])
                        yield 1700.0
                        psk = qkmm(wk_sb, 1)
                        rope(psk, kT[b][1])
                        yield 1700.0
                    else:
                        for w_sb, dsts in ((wq_sb, qT[b]), (wk_sb, kT[b])):
                            for hh in range(HPC):
                                rope(qkmm(w_sb, hh), dsts[hh])
                                yield 1700.0
                    if b == 0 and ti == 2:
                        nc.sync.dma_start(cs_sb[:, 3 * TQ:],
                                          cs2.ap()[:, 3 * TQ:])
                        nc.sync.dma_start(sn_sb[:, 3 * TQ:],
                                          sn1.ap()[:, 3 * TQ:])
                        nc.sync.dma_start(tri_sb[:], tri.ap())
                    # prefetch next x tile before the v-section so its DMA
                    # gets ahead of lower-priority queue entries
                    nb, nti = (b, ti + 1) if ti + 1 < NT else (b + 1, 0)
                    if nb < B and (nb, nti) > state.get("pref", (-1, -1)):
                        xtn = xp.tile([P, KO, TQ], bf16, tag="xt",
                                      name=f"xt{nb}_{nti}")
                        nc.sync.dma_start(xtn[:], xTt.ap()[nb, nti])
                        state["next"] = xtn
                        state["pref"] = (nb, nti)
                    if b == 0 and ti == 0:
                        nc.sync.dma_start(cs_sb[:, TQ:3 * TQ],
                                          cs2.ap()[:, TQ:3 * TQ])
                        nc.sync.dma_start(sn_sb[:, TQ:3 * TQ],
                                          sn1.ap()[:, TQ:3 * TQ])
                    for sub in range(TQ // P):
                        psv = psa.tile([P, HPC * D], f32, tag="acc",
                                       name=f"accv{b}_{ti}_{sub}")
                        for ko in range(KO):
                            nc.tensor.matmul(
                                psv[:], xt[:, ko, sub * P:(sub + 1) * P],
                                wv_sb[:, ko, :],
                                start=(ko == 0), stop=(ko == KO - 1))
                        tci = ti * (TQ // P) + sub
                        for hh in range(HPC):
                            nc.vector.tensor_copy(
                                vt[b][hh][:, tci, :],
                                psv[:, hh * D:(hh + 1) * D])
                        yield 1700.0

            def gen_attn(b):
                # Both heads interleaved (probs packed in one [P,2,TA] tile
                # so their denominator matmuls merge); o/sum matmuls trail
                # score/exp by one iteration so the PE isn't chained to the
                # Exp latency.  Mask/normalize run on gpsimd -- the vector
                # queue is busy with RoPE and its latency would stall the PE.
                for ta in range(NTA):
                    ps_o = [pso.tile([P, TA], f32, tag=f"o{hh}",
                                     name=f"o{b}_{ta}_{hh}")
                            for hh in range(HPC)]
                    ps_sum = pssum.tile([P, HPC, TA], f32, tag="sum",
                                        name=f"sum{b}_{ta}")
                    smax = (ta + 1) * SCPT - 1
                    pending = []

                    def drain(n, _p=pending, _o=ps_o, _s=ps_sum, _m=smax):
                        while len(_p) > n:
                            s, w, pt2 = _p.pop(0)
                            first, last = (s == 0), (s == _m)
                            for hh in range(HPC):
                                nc.tensor.matmul(_o[hh][:, w],
                                                 vt[b][hh][:, s, :],
                                                 pt2[:, hh, w],
                                                 start=first, stop=last)
                                nc.tensor.matmul(_s[:, hh, w],
                                                 ones_mat[:], pt2[:, hh, w],
                                                 start=first, stop=last)

                    for s in range(smax + 1):
                        j = s - ta * SCPT  # >=0 on the diagonal
                        w0 = P * max(j, 0)
                        w = slice(w0, TA)
                        qsl = slice(ta * TA + w0, (ta + 1) * TA)
                        pt2 = ptp.tile([P, HPC, TA], bf16, tag="pt",
                                       name=f"pt{b}_{ta}_{s}")
                        for hh in range(HPC):
                            ps_sc = pssc.tile([P, TA], f32, tag="sc",
                                              name=f"sc{b}_{ta}_{s}_{hh}")
                            nc.tensor.matmul(
                                ps_sc[:, w], kT[b][hh][:, s * P:(s + 1) * P],
                                qT[b][hh][:, qsl], start=True, stop=True)
                            nc.scalar.activation(
                                pt2[:, hh, w], ps_sc[:, w],
                                mybir.ActivationFunctionType.Exp,
                                scale=SCALE)
                            if j >= 0:  # mask the 128x128 triangle
                                nc.gpsimd.tensor_mul(
                                    pt2[:, hh, w0:w0 + P],
                                    pt2[:, hh, w0:w0 + P], tri_sb[:])
                        pending.append((s, w, pt2))
                        drain(2)
                        yield 5 * (TA - w0) * 0.42
                    drain(0)
                    for hh in range(HPC):
                        recf = sp.tile([P, TA], f32, tag="recf")
                        nc.vector.reciprocal_approx_fast(
                            recf[:], ps_sum[:, hh, :])
                        o_sb = sp.tile([P, TA], f32, tag="o_sb")
                        nc.scalar.copy(o_sb[:], ps_o[hh][:])
                        # write oT pre-shuffled for the out-projection:
                        # oT[p, j*128+u] = O^T[p, t=u*16+j]
                        oview = oT[b][hh].rearrange(
                            "p (j u) -> p u j", j=KO)[
                            :, (TA // 16) * ta:(TA // 16) * (ta + 1), :]
                        nc.gpsimd.tensor_mul(
                            oview,
                            o_sb[:].rearrange("p (u j) -> p u j", j=KO),
                            recf[:].rearrange("p (u j) -> p u j", j=KO))
                    yield 500.0

            def wload(b, cpi):
                wcp = wop.tile([P, KO, TC_], bf16, tag="w",
                               name=f"w{b}_{cpi}")
                nc.sync.dma_start(wcp[:], woutp.ap()[cpi])
                state_w[(b, cpi)] = wcp

            def gen_out(b, psc, cpis=None):
                for cpi in (cpis if cpis is not None else range(NCP)):
                    csl = slice(cpi * TC_, (cpi + 1) * TC_)
                    if (b, cpi) not in state_w:
                        wload(b, cpi)
                    wcp = state_w[(b, cpi)]
                    if b == 0 and cpi == NCP - 1:
                        # prefetch the b=1 pass's first tiles; their slot
                        # WARs resolve as this pass's groups retire.
                        for nc_ in range(4):
                            wload(1, nc_)
                    for hh in range(HPC):
                        psy = psc.tile([P, TC_], f32, tag="y",
                                       name=f"y{b}_{cpi}_{hh}")
                        for j in range(KO):
                            nc.tensor.matmul(psy[:],
                                             oT[b][hh][:, j * P:(j + 1) * P],
                                             wcp[:, j, :],
                                             start=(j == 0),
                                             stop=(j == KO - 1))
                        ysb = sp.tile([P, TC_], f32, tag="ysb")
                        if b == 1:
                            nc.scalar.copy(ysb[:], psy[:])
                        else:
                            nc.vector.tensor_copy(ysb[:], psy[:])
                        nc.sync.dma_start(
                            y.ap()[b, hh * D:(hh + 1) * D, csl], ysb[:])
                        yield 1700.0

            state_w = {}
            qstate = {}
            with tc.tile_pool(name="xt", bufs=2) as xp, \
                 tc.tile_pool(name="rope", bufs=3) as rp, \
                 tc.tile_pool(name="psA", bufs=2, space="PSUM") as psa:
                # phase A: qkv(b=0) alone
                for _ in gen_qkv(0, xp, psa, rp, qstate):
                    pass
                # phase B: attention(b=0) interleaved with qkv(b=1)
                _merge(gen_attn(0), gen_qkv(1, xp, psa, rp, qstate))
                wload(0, 0)
                wload(0, 1)
            with tc.tile_pool(name="psC", bufs=2, space="PSUM") as psc:
                # phase C: attention(b=1) interleaved with out-proj(b=0);
                # the last two b=0 column tiles are held back to bridge the
                # C->D transition while oT[b=1] finishes normalizing.
                _merge(gen_attn(1), gen_out(0, psc, range(NCP - 2)))
                for _ in gen_out(0, psc, range(NCP - 2, NCP)):
                    pass
                # phase D: out-proj(b=1)
                for _ in gen_out(1, psc):
                    pass

    nc.compile()
    return nc


_NC = None


def _get_nc():
    global _NC
    if _NC is None:
        _NC = _build()
    return _NC


def _host_tables():
    pos = np.arange(T, dtype=np.float32)[:, None]
    div = np.exp(np.arange(0, 2 * HALF, 2, dtype=np.float32)
                 * np.float32(-math.log(ROPE_BASE) / (2 * HALF)))
    ang = pos * div[None, :]
    cosv = np.cos(ang).astype(np.float32)   # [T, HALF]
    sinv = np.sin(ang).astype(np.float32)
    cosT = np.ascontiguousarray(cosv.T)     # [HALF, T]
    sinT = np.ascontiguousarray(sinv.T)
    cs2 = np.ascontiguousarray(np.concatenate([cosT, cosT], axis=0))  # [P, T]
    sn1 = sinT
    # triangle mask tri[s, u] = 1 iff s <= u
    uu = np.arange(P)[None, :]
    ss = np.arange(P)[:, None]
    trim = (ss <= uu).astype(ml_dtypes.bfloat16)
    return cs2, sn1, trim


def _make_in_maps(x, Wqkv, Wout):
    x = np.asarray(x, dtype=np.float32)
    Wqkv = np.asarray(Wqkv, dtype=np.float32)
    Wout = np.asarray(Wout, dtype=np.float32)
    assert x.shape == (B, T, C) and Wqkv.shape == (C, 3 * C) \
        and Wout.shape == (C, C)

    cs2, sn1, trim = _host_tables()
    # xTt[b, ti, p, ko, u] = x[b, ti*TQ+u, ko*128+p]
    xTt = np.ascontiguousarray(
        x.astype(ml_dtypes.bfloat16)
        .reshape(B, NT, TQ, KO, P).transpose(0, 1, 4, 3, 2))
    # woutp[cpi, p, j, m] = Wout[j*128+p, cpi*TC_+m]
    woutp = np.ascontiguousarray(
        Wout.astype(ml_dtypes.bfloat16).reshape(KO, P, NCP, TC_)
        .transpose(2, 1, 0, 3))

    in_maps = []
    for core in range(NCORES):
        h0 = core * HPC
        cols = slice(h0 * D, (h0 + HPC) * D)
        ws = []
        for part in range(3):
            w = Wqkv[:, part * C:(part + 1) * C][:, cols] \
                .astype(ml_dtypes.bfloat16)  # [C, HPC*D]
            if part < 2:  # wq/wk: [P, HPC, KO, D]
                ws.append(np.ascontiguousarray(
                    w.reshape(KO, P, HPC, D).transpose(1, 2, 0, 3)))
            else:         # wv: [P, KO, HPC*D]
                ws.append(np.ascontiguousarray(
                    w.reshape(KO, P, HPC * D).transpose(1, 0, 2)))
        in_maps.append({
            "xTt": xTt,
            "wq": ws[0], "wk": ws[1], "wv": ws[2],
            "woutp": woutp,
            "cs2": cs2, "sn1": sn1, "tri": trim,
        })
    return in_maps


def _run(x, Wqkv, Wout, trace=False):
    nc = _get_nc()
    in_maps = _make_in_maps(x, Wqkv, Wout)
    res = run_bass_kernel_spmd(nc, in_maps, core_ids=list(range(NCORES)),
                               trace=trace)
    out = np.empty((B, T, C), dtype=np.float32)
    for core in range(NCORES):
        out[:, core * HPC * D:(core + 1) * HPC * D, :] = \
            res.results[core]["y"]
    return out, res


def kernel(x, Wqkv, Wout):
    out, _ = _run(x, Wqkv, Wout)
    return out


# revision 28
# speedup vs baseline: 1.0041x; 1.0041x over previous
"""Trainium2 Bass kernel for nn_MultiHeadAttention_63015760167496.

Computation (see reference): qkv = x @ Wqkv; RoPE on q,k; causal softmax
attention per head; out = einsum('bhts,bshd->bhtd', probs, v);
out.reshape(B,T,C) @ Wout  -- NOTE the reshape is a *head-major* flatten of
[B,H,T,D] into [B,T,C], so final-output row r = h*128 + t//16 depends only on
head h.  Sharding: head-parallel over 8 cores (2 heads/core); every core
computes its two heads end-to-end and produces final-output rows
[256*i, 256*i+256).  Host concatenates -- no collectives.

The QKV projection runs as float32r (TF32-like, 1 col/cycle at N>=256).
q/k/v (post-RoPE), attention probabilities, attention outputs and Wout are
bf16: same PE rate with no fp32r N>=256 restriction, 2x DVE throughput for
sbuf-only elementwise ops, half the Wout DMA.  Attention uses the S^T
layout ([s,t]): softmax denominator via a ones-vector matmul (partition
reduction on the PE), normalization via a K=1 broadcast matmul.  No running
max is needed (scores are O(5), fp32 psum).

The TRN2 PE clock ramps (0.65 -> 1.2 -> 2.4 GHz) only under *continuous*
load; any idle gap drops it back.  A bare attention phase is paced by the
scalar-engine Exp and keeps micro-stalling the PE, which then runs its
matmuls at roughly half clock.  So the schedule interleaves phases that
have independent tensor work:
    A: qkv(b=0)
    B: attention(b=0) + qkv(b=1)        (interleaved emission)
    C: attention(b=1) + out-proj(b=0)   (interleaved emission)
    D: out-proj(b=1)
The interleaving is done by a cost-weighted round-robin merge of per-phase
emission generators; the Tile framework's semaphores keep it correct.
"""

import math
import sys

for _p in ("/opt/trn_rl_repo", "/root/.axon_site/_ro/trn_rl_repo"):
    if _p not in sys.path:
        sys.path.insert(0, _p)

import numpy as np
import ml_dtypes

import concourse.bass as bass
import concourse.mybir as mybir
import concourse.tile as tile
from concourse import bacc
from concourse.bass_utils import run_bass_kernel_spmd

B, T, C = 2, 2048, 2048
H = 16            # heads total
D = C // H        # 128 head dim
HALF = D // 2     # 64
P = 128
KO = C // P       # 16 contraction chunks
NCORES = 8
HPC = H // NCORES  # 2 heads per core
TQ = 256          # t-tile for qkv projection
NT = T // TQ
TA = 512          # t-tile for attention
NTA = T // TA
SCPT = TA // P    # 4 s-chunks per attention tile
TC_ = 256         # out-projection column tile
NCP = C // TC_
ROPE_BASE = 10000.0
SCALE = 1.0 / math.sqrt(D)

f32 = mybir.dt.float32
f32r = mybir.dt.float32r
bf16 = mybir.dt.bfloat16


def _merge(*gens):
    """Cost-weighted round-robin: always step the generator with the least
    accumulated emitted-tensor-time.  Generators yield ns estimates."""
    acc = [0.0] * len(gens)
    live = list(range(len(gens)))
    while live:
        i = min(live, key=lambda k: acc[k])
        try:
            acc[i] += next(gens[i])
        except StopIteration:
            live.remove(i)


def _build():
    nc = bacc.Bacc("TRN2", target_bir_lowering=False, debug=False,
                   num_devices=NCORES)

    # host-pre-tiled x^T: xTt[b, ti, p, ko, u] = x[b, ti*TQ+u, ko*128+p]
    xTt = nc.dram_tensor("xTt", [B, NT, P, KO, TQ], bf16, kind="ExternalInput")
    # host-pre-chunked weights: wq/wk[p, hh, ko, d] = W[ko*128+p, hh*128+d]
    wq = nc.dram_tensor("wq", [P, HPC, KO, D], bf16, kind="ExternalInput")
    wk = nc.dram_tensor("wk", [P, HPC, KO, D], bf16, kind="ExternalInput")
    wv = nc.dram_tensor("wv", [P, KO, HPC * D], bf16, kind="ExternalInput")
    # woutp[cpi, p, j, m] = Wout[j*128+p, cpi*TC_+m], bf16
    woutp = nc.dram_tensor("woutp", [NCP, P, KO, TC_], bf16,
                           kind="ExternalInput")
    cs2 = nc.dram_tensor("cs2", [P, T], f32, kind="ExternalInput")  # [cos;cos]
    sn1 = nc.dram_tensor("sn1", [HALF, T], f32, kind="ExternalInput")  # sin
    # tri[s, u] = 1 iff s <= u (valid upper triangle in the S^T layout)
    tri = nc.dram_tensor("tri", [P, P], bf16, kind="ExternalInput")
    y = nc.dram_tensor("y", [B, HPC * D, C], f32, kind="ExternalOutput")

    with tile.TileContext(nc) as tc:
        with tc.tile_pool(name="const", bufs=1) as cp_, \
             tc.tile_pool(name="qkv", bufs=1) as qp, \
             tc.tile_pool(name="ot", bufs=1) as op_, \
             tc.tile_pool(name="wo", bufs=4) as wop, \
             tc.tile_pool(name="small", bufs=3) as sp, \
             tc.tile_pool(name="pt", bufs=6) as ptp, \
             tc.tile_pool(name="psBsc", bufs=2, space="PSUM") as pssc, \
             tc.tile_pool(name="psBo", bufs=1, space="PSUM") as pso, \
             tc.tile_pool(name="psBsum", bufs=1, space="PSUM") as pssum:

            wq_sb = cp_.tile([P, HPC, KO, D], bf16, tag="wq")
            wk_sb = cp_.tile([P, HPC, KO, D], bf16, tag="wk")
            wv_sb = cp_.tile([P, KO, HPC * D], bf16, tag="wv")
            # first q matmuls need only wq[:, 0]; split the DMA so they
            # start sooner.  cs/sn head chunks come first: RoPE(ti=0) gates
            # the psum-accumulator recycling.
            nc.sync.dma_start(wq_sb[:, 0], wq.ap()[:, 0])
            cs_sb = cp_.tile([P, T], f32, tag="cs")
            sn_sb = cp_.tile([HALF, T], f32, tag="sn")
            tri_sb = cp_.tile([P, P], bf16, tag="tri")
            # ones *matrix* stationary for the denominator matmuls: same
            # cycle cost as a ones-vector (cost ~ moving size), but keeps
            # the PE tile config at (128,128) -- a (128,32) sum config
            # between the score/o matmuls breaks ldweights pipelining --
            # and broadcasts the sums to all partitions, which makes the
            # normalization a plain elementwise multiply (no K=1 broadcast
            # matmul needed).
            ones_f32 = cp_.tile([P, P], f32, tag="ones_f32")
            nc.vector.memset(ones_f32[:], 1.0)
            ones_mat = cp_.tile([P, P], bf16, tag="ones_mat")
            nc.vector.tensor_copy(ones_mat[:], ones_f32[:])

            # PE warmup: the clock ramp (0.65->2.4GHz) needs sustained
            # activity; burn cheap dummy matmuls while the first DMAs land.
            warm_sb = cp_.tile([P, TQ], bf16, tag="warm_sb")
            nc.vector.memset(warm_sb[:], 0.0)
            ps_warm = pssc.tile([P, TA], f32, tag="sc", name="warm")
            for wi in range(6):
                nc.tensor.matmul(ps_warm[:, 0:TQ], ones_mat[:], warm_sb[:],
                                 start=True, stop=True)

            # persistent attention outputs O^T per (b, local head): [d, t]
            oT = [[op_.tile([P, T], bf16, tag=f"oT{b}{hh}", name=f"oT{b}{hh}")
                   for hh in range(HPC)] for b in range(B)]
            qT = [[qp.tile([P, T], bf16, tag=f"qT{b}{hh}", name=f"qT{b}{hh}")
                   for hh in range(HPC)] for b in range(B)]
            kT = [[qp.tile([P, T], bf16, tag=f"kT{b}{hh}", name=f"kT{b}{hh}")
                   for hh in range(HPC)] for b in range(B)]
            vt = [[qp.tile([P, T // P, D], bf16, tag=f"v{b}{hh}",
                           name=f"v{b}{hh}")
                   for hh in range(HPC)] for b in range(B)]

            def gen_qkv(b, xp, psa, rp, state):
                for ti in range(NT):
                    sl = slice(ti * TQ, (ti + 1) * TQ)
                    first_tile = (b == 0 and ti == 0)
                    if state.get("next") is not None:
                        xt = state.pop("next")
                    else:
                        xt = xp.tile([P, KO, TQ], bf16, tag="xt",
                                     name=f"xt{b}_{ti}")
                        if not first_tile:
                            nc.sync.dma_start(xt[:], xTt.ap()[b, ti])
                    cs = cs_sb[:, sl]
                    sn = sn_sb[:, sl]  # [64, TQ] base partition 0

                    def qkmm(w_sb, hh, split_dma=False, csn_after=None):
                        ps = psa.tile([P, TQ], f32, tag="acc",
                                      name=f"acc{b}_{ti}_{hh}")
                        for ko in range(KO):
                            if split_dma and ko % 2 == 0:
                                kos = slice(ko, ko + 2)
                                nc.sync.dma_start(xt[:, kos],
                                                  xTt.ap()[b, ti, :, kos])
                                if ko == 4 and csn_after:
                                    nc.sync.dma_start(cs_sb[:, 0:TQ],
                                                      cs2.ap()[:, 0:TQ])
                                    nc.sync.dma_start(sn_sb[:, 0:TQ],
                                                      sn1.ap()[:, 0:TQ])
                            nc.tensor.matmul(ps[:], w_sb[:, hh, ko, :],
                                             xt[:, ko, :],
                                             start=(ko == 0),
                                             stop=(ko == KO - 1))
                        return ps

                    def rope(ps, dst):
                        # tcos = ps * [cos;cos] (one full mult); tsw
                        # pre-swaps halves: tsw[0:64]=q2*sin,
                        # tsw[64:128]=q1*sin so the gpsimd add/sub reads
                        # align on base partitions.
                        tcos = rp.tile([P, TQ], f32, tag="tcos")
                        tsw = rp.tile([P, TQ], f32, tag="tsw")
                        nc.vector.tensor_mul(tcos[:], ps[:], cs)
                        nc.vector.tensor_mul(tsw[0:HALF, :],
                                             ps[HALF:P, :], sn)
                        nc.vector.tensor_mul(tsw[HALF:P, :],
                                             ps[0:HALF, :], sn)
                        nc.gpsimd.tensor_sub(dst[0:HALF, sl],
                                             tcos[0:HALF, :],
                                             tsw[0:HALF, :])
                        nc.gpsimd.tensor_add(dst[HALF:P, sl],
                                             tcos[HALF:P, :],
                                             tsw[HALF:P, :])

                    if first_tile:
                        # q accums first (need only wq + xt0, DMA'd in ko
                        # quarters); stagger the remaining const DMAs
                        # behind them.
                        ps0 = qkmm(wq_sb, 0, split_dma=True,
                                   csn_after=1)
                        nc.sync.dma_start(wq_sb[:, 1], wq.ap()[:, 1])
                        nc.sync.dma_start(wk_sb[:], wk.ap())
                        yield 1700.0
                        ps1 = qkmm(wq_sb, 1)
                        nc.sync.dma_start(wv_sb[:], wv.ap())
                        rope(ps0, qT[b][0])
                        yield 1700.0
                        psk = qkmm(wk_sb, 0)
                        rope(ps1, qT[b][1])
                        rope(psk, kT[b][0])
                        yield 1700.0
                        psk = qkmm(wk_sb, 1)
                        rope(psk, kT[b][1])
                        yield 1700.0
                    else:
                        for w_sb, dsts in ((wq_sb, qT[b]), (wk_sb, kT[b])):
                            for hh in range(HPC):
                                rope(qkmm(w_sb, hh), dsts[hh])
                                yield 1700.0
                    if b == 0 and ti == 2:
                        nc.sync.dma_start(cs_sb[:, 3 * TQ:],
                                          cs2.ap()[:, 3 * TQ:])
                        nc.sync.dma_start(sn_sb[:, 3 * TQ:],
                                          sn1.ap()[:, 3 * TQ:])
                        nc.sync.dma_start(tri_sb[:], tri.ap())
                    # prefetch next x tile before the v-section so its DMA
                    # gets ahead of lower-priority queue entries
                    nb, nti = (b, ti + 1) if ti + 1 < NT else (b + 1, 0)
                    if nb < B and (nb, nti) > state.get("pref", (-1, -1)):
                        xtn = xp.tile([P, KO, TQ], bf16, tag="xt",
                                      name=f"xt{nb}_{nti}")
                        nc.sync.dma_start(xtn[:], xTt.ap()[nb, nti])
                        state["next"] = xtn
                        state["pref"] = (nb, nti)
                    if b == 0 and ti == 0:
                        nc.sync.dma_start(cs_sb[:, TQ:3 * TQ],
                                          cs2.ap()[:, TQ:3 * TQ])
                        nc.sync.dma_start(sn_sb[:, TQ:3 * TQ],
                                          sn1.ap()[:, TQ:3 * TQ])
                    for sub in range(TQ // P):
                        psv = psa.tile([P, HPC * D], f32, tag="acc",
                                       name=f"accv{b}_{ti}_{sub}")
                        for ko in range(KO):
                            nc.tensor.matmul(
                                psv[:], xt[:, ko, sub * P:(sub + 1) * P],
                                wv_sb[:, ko, :],
                                start=(ko == 0), stop=(ko == KO - 1))
                        tci = ti * (TQ // P) + sub
                        for hh in range(HPC):
                            nc.vector.tensor_copy(
                                vt[b][hh][:, tci, :],
                                psv[:, hh * D:(hh + 1) * D])
                        yield 1700.0

            def gen_attn(b):
                # Both heads interleaved (probs packed in one [P,2,TA] tile
                # so their denominator matmuls merge); o/sum matmuls trail
                # score/exp by one iteration so the PE isn't chained to the
                # Exp latency.  Mask/normalize run on gpsimd -- the vector
                # queue is busy with RoPE and its latency would stall the PE.
                for ta in range(NTA):
                    ps_o = [pso.tile([P, TA], f32, tag=f"o{hh}",
                                     name=f"o{b}_{ta}_{hh}")
                            for hh in range(HPC)]
                    ps_sum = pssum.tile([P, HPC, TA], f32, tag="sum",
                                        name=f"sum{b}_{ta}")
                    smax = (ta + 1) * SCPT - 1
                    pending = []

                    def drain(n, _p=pending, _o=ps_o, _s=ps_sum, _m=smax):
                        while len(_p) > n:
                            s, w, pt2 = _p.pop(0)
                            first, last = (s == 0), (s == _m)
                            for hh in range(HPC):
                                nc.tensor.matmul(_o[hh][:, w],
                                                 vt[b][hh][:, s, :],
                                                 pt2[:, hh, w],
                                                 start=first, stop=last)
                                nc.tensor.matmul(_s[:, hh, w],
                                                 ones_mat[:], pt2[:, hh, w],
                                                 start=first, stop=last)

                    for s in range(smax + 1):
                        j = s - ta * SCPT  # >=0 on the diagonal
                        w0 = P * max(j, 0)
                        w = slice(w0, TA)
                        qsl = slice(ta * TA + w0, (ta + 1) * TA)
                        pt2 = ptp.tile([P, HPC, TA], bf16, tag="pt",
                                       name=f"pt{b}_{ta}_{s}")
                        for hh in range(HPC):
                            ps_sc = pssc.tile([P, TA], f32, tag="sc",
                                              name=f"sc{b}_{ta}_{s}_{hh}")
                            nc.tensor.matmul(
                                ps_sc[:, w], kT[b][hh][:, s * P:(s + 1) * P],
                                qT[b][hh][:, qsl], start=True, stop=True)
                            nc.scalar.activation(
                                pt2[:, hh, w], ps_sc[:, w],
                                mybir.ActivationFunctionType.Exp,
                                scale=SCALE)
                            if j >= 0:  # mask the 128x128 triangle
                                nc.gpsimd.tensor_mul(
                                    pt2[:, hh, w0:w0 + P],
                                    pt2[:, hh, w0:w0 + P], tri_sb[:])
                        pending.append((s, w, pt2))
                        drain(2)
                        yield 5 * (TA - w0) * 0.42
                    drain(0)
                    for hh in range(HPC):
                        recf = sp.tile([P, TA], f32, tag="recf")
                        nc.vector.reciprocal_approx_fast(
                            recf[:], ps_sum[:, hh, :])
                        o_sb = sp.tile([P, TA], f32, tag="o_sb")
                        nc.scalar.copy(o_sb[:], ps_o[hh][:])
                        # write oT pre-shuffled for the out-projection:
                        # oT[p, j*128+u] = O^T[p, t=u*16+j]
                        oview = oT[b][hh].rearrange(
                            "p (j u) -> p u j", j=KO)[
                            :, (TA // 16) * ta:(TA // 16) * (ta + 1), :]
                        nc.gpsimd.tensor_mul(
                            oview,
                            o_sb[:].rearrange("p (u j) -> p u j", j=KO),
                            recf[:].rearrange("p (u j) -> p u j", j=KO))
                    yield 500.0

            def wload(b, cpi):
                wcp = wop.tile([P, KO, TC_], bf16, tag="w",
                               name=f"w{b}_{cpi}")
                nc.sync.dma_start(wcp[:], woutp.ap()[cpi])
                state_w[(b, cpi)] = wcp

            def gen_out(b, psc, cpis=None):
                for cpi in (cpis if cpis is not None else range(NCP)):
                    csl = slice(cpi * TC_, (cpi + 1) * TC_)
                    if (b, cpi) not in state_w:
                        wload(b, cpi)
                    wcp = state_w[(b, cpi)]
                    if b == 0 and cpi == NCP - 1:
                        # prefetch the b=1 pass's first tiles; their slot
                        # WARs resolve as this pass's groups retire.
                        for nc_ in range(4):
                            wload(1, nc_)
                    for hh in range(HPC):
                        psy = psc.tile([P, TC_], f32, tag="y",
                                       name=f"y{b}_{cpi}_{hh}")
                        for j in range(KO):
                            nc.tensor.matmul(psy[:],
                                             oT[b][hh][:, j * P:(j + 1) * P],
                                             wcp[:, j, :],
                                             start=(j == 0),
                                             stop=(j == KO - 1))
                        ysb = sp.tile([P, TC_], f32, tag="ysb")
                        if b == 1:
                            nc.scalar.copy(ysb[:], psy[:])
                        else:
                            nc.vector.tensor_copy(ysb[:], psy[:])
                        nc.sync.dma_start(
                            y.ap()[b, hh * D:(hh + 1) * D, csl], ysb[:])
                        yield 1700.0

            state_w = {}
            qstate = {}
            with tc.tile_pool(name="xt", bufs=2) as xp, \
                 tc.tile_pool(name="rope", bufs=3) as rp, \
                 tc.tile_pool(name="psA", bufs=2, space="PSUM") as psa:
                # phase A: qkv(b=0) alone
                for _ in gen_qkv(0, xp, psa, rp, qstate):
                    pass
                # phase B: attention(b=0) interleaved with qkv(b=1)
                _merge(gen_attn(0), gen_qkv(1, xp, psa, rp, qstate))
                wload(0, 0)
                wload(0, 1)
            with tc.tile_pool(name="psC", bufs=2, space="PSUM") as psc:
                # phase C: attention(b=1) interleaved with out-proj(b=0);
                # the last two b=0 column tiles are held back to bridge the
                # C->D transition while oT[b=1] finishes normalizing.
                _merge(gen_attn(1), gen_out(0, psc, range(NCP - 2)))
                for _ in gen_out(0, psc, range(NCP - 2, NCP)):
                    pass
                # phase D: out-proj(b=1)
                for _ in gen_out(1, psc):
                    pass

    nc.compile()
    return nc


_NC = None


def _get_nc():
    global _NC
    if _NC is None:
        _NC = _build()
    return _NC


def _host_tables():
    pos = np.arange(T, dtype=np.float32)[:, None]
    div = np.exp(np.arange(0, 2 * HALF, 2, dtype=np.float32)
                 * np.float32(-math.log(ROPE_BASE) / (2 * HALF)))
    ang = pos * div[None, :]
    cosv = np.cos(ang).astype(np.float32)   # [T, HALF]
    sinv = np.sin(ang).astype(np.float32)
    cosT = np.ascontiguousarray(cosv.T)     # [HALF, T]
    sinT = np.ascontiguousarray(sinv.T)
    cs2 = np.ascontiguousarray(np.concatenate([cosT, cosT], axis=0))  # [P, T]
    sn1 = sinT
    # triangle mask tri[s, u] = 1 iff s <= u
    uu = np.arange(P)[None, :]
    ss = np.arange(P)[:, None]
    trim = (ss <= uu).astype(ml_dtypes.bfloat16)
    return cs2, sn1, trim


def _make_in_maps(x, Wqkv, Wout):
    x = np.asarray(x, dtype=np.float32)
    Wqkv = np.asarray(Wqkv, dtype=np.float32)
    Wout = np.asarray(Wout, dtype=np.float32)
    assert x.shape == (B, T, C) and Wqkv.shape == (C, 3 * C) \
        and Wout.shape == (C, C)

    cs2, sn1, trim = _host_tables()
    # xTt[b, ti, p, ko, u] = x[b, ti*TQ+u, ko*128+p]
    xTt = np.ascontiguousarray(
        x.astype(ml_dtypes.bfloat16)
        .reshape(B, NT, TQ, KO, P).transpose(0, 1, 4, 3, 2))
    # woutp[cpi, p, j, m] = Wout[j*128+p, cpi*TC_+m]
    woutp = np.ascontiguousarray(
        Wout.astype(ml_dtypes.bfloat16).reshape(KO, P, NCP, TC_)
        .transpose(2, 1, 0, 3))

    in_maps = []
    for core in range(NCORES):
        h0 = core * HPC
        cols = slice(h0 * D, (h0 + HPC) * D)
        ws = []
        for part in range(3):
            w = Wqkv[:, part * C:(part + 1) * C][:, cols] \
                .astype(ml_dtypes.bfloat16)  # [C, HPC*D]
            if part < 2:  # wq/wk: [P, HPC, KO, D]
                ws.append(np.ascontiguousarray(
                    w.reshape(KO, P, HPC, D).transpose(1, 2, 0, 3)))
            else:         # wv: [P, KO, HPC*D]
                ws.append(np.ascontiguousarray(
                    w.reshape(KO, P, HPC * D).transpose(1, 0, 2)))
        in_maps.append({
            "xTt": xTt,
            "wq": ws[0], "wk": ws[1], "wv": ws[2],
            "woutp": woutp,
            "cs2": cs2, "sn1": sn1, "tri": trim,
        })
    return in_maps


def _run(x, Wqkv, Wout, trace=False):
    nc = _get_nc()
    in_maps = _make_in_maps(x, Wqkv, Wout)
    res = run_bass_kernel_spmd(nc, in_maps, core_ids=list(range(NCORES)),
                               trace=trace)
    out = np.empty((B, T, C), dtype=np.float32)
    for core in range(NCORES):
        out[:, core * HPC * D:(core + 1) * HPC * D, :] = \
            res.results[core]["y"]
    return out, res


def kernel(x, Wqkv, Wout):
    out, _ = _run(x, Wqkv, Wout)
    return out


# revision 29
# speedup vs baseline: 1.0100x; 1.0059x over previous
"""Trainium2 Bass kernel for nn_MultiHeadAttention_63015760167496.

Computation (see reference): qkv = x @ Wqkv; RoPE on q,k; causal softmax
attention per head; out = einsum('bhts,bshd->bhtd', probs, v);
out.reshape(B,T,C) @ Wout  -- NOTE the reshape is a *head-major* flatten of
[B,H,T,D] into [B,T,C], so final-output row r = h*128 + t//16 depends only on
head h.  Sharding: head-parallel over 8 cores (2 heads/core); every core
computes its two heads end-to-end and produces final-output rows
[256*i, 256*i+256).  Host concatenates -- no collectives.

The QKV projection runs as float32r (TF32-like, 1 col/cycle at N>=256).
q/k/v (post-RoPE), attention probabilities, attention outputs and Wout are
bf16: same PE rate with no fp32r N>=256 restriction, 2x DVE throughput for
sbuf-only elementwise ops, half the Wout DMA.  Attention uses the S^T
layout ([s,t]): softmax denominator via a ones-vector matmul (partition
reduction on the PE), normalization via a K=1 broadcast matmul.  No running
max is needed (scores are O(5), fp32 psum).

The TRN2 PE clock ramps (0.65 -> 1.2 -> 2.4 GHz) only under *continuous*
load; any idle gap drops it back.  A bare attention phase is paced by the
scalar-engine Exp and keeps micro-stalling the PE, which then runs its
matmuls at roughly half clock.  So the schedule interleaves phases that
have independent tensor work:
    A: qkv(b=0)
    B: attention(b=0) + qkv(b=1)        (interleaved emission)
    C: attention(b=1) + out-proj(b=0)   (interleaved emission)
    D: out-proj(b=1)
The interleaving is done by a cost-weighted round-robin merge of per-phase
emission generators; the Tile framework's semaphores keep it correct.
"""

import math
import sys

for _p in ("/opt/trn_rl_repo", "/root/.axon_site/_ro/trn_rl_repo"):
    if _p not in sys.path:
        sys.path.insert(0, _p)

import numpy as np
import ml_dtypes

import concourse.bass as bass
import concourse.mybir as mybir
import concourse.tile as tile
from concourse import bacc
from concourse.bass_utils import run_bass_kernel_spmd

B, T, C = 2, 2048, 2048
H = 16            # heads total
D = C // H        # 128 head dim
HALF = D // 2     # 64
P = 128
KO = C // P       # 16 contraction chunks
NCORES = 8
HPC = H // NCORES  # 2 heads per core
TQ = 256          # t-tile for qkv projection
NT = T // TQ
TA = 512          # t-tile for attention
NTA = T // TA
SCPT = TA // P    # 4 s-chunks per attention tile
TC_ = 256         # out-projection column tile
NCP = C // TC_
ROPE_BASE = 10000.0
SCALE = 1.0 / math.sqrt(D)

f32 = mybir.dt.float32
f32r = mybir.dt.float32r
bf16 = mybir.dt.bfloat16


def _merge(*gens):
    """Cost-weighted round-robin: always step the generator with the least
    accumulated emitted-tensor-time.  Generators yield ns estimates."""
    acc = [0.0] * len(gens)
    live = list(range(len(gens)))
    while live:
        i = min(live, key=lambda k: acc[k])
        try:
            acc[i] += next(gens[i])
        except StopIteration:
            live.remove(i)


def _build():
    nc = bacc.Bacc("TRN2", target_bir_lowering=False, debug=False,
                   num_devices=NCORES)

    # host-pre-tiled x^T: xTt[b, ti, p, ko, u] = x[b, ti*TQ+u, ko*128+p]
    xTt = nc.dram_tensor("xTt", [B, NT, P, KO, TQ], bf16, kind="ExternalInput")
    # host-pre-chunked weights: wq/wk[p, hh, ko, d] = W[ko*128+p, hh*128+d]
    wq = nc.dram_tensor("wq", [P, HPC, KO, D], bf16, kind="ExternalInput")
    wk = nc.dram_tensor("wk", [P, HPC, KO, D], bf16, kind="ExternalInput")
    wv = nc.dram_tensor("wv", [P, KO, HPC * D], bf16, kind="ExternalInput")
    # woutp[cpi, p, j, m] = Wout[j*128+p, cpi*TC_+m], bf16
    woutp = nc.dram_tensor("woutp", [NCP, P, KO, TC_], bf16,
                           kind="ExternalInput")
    cs2 = nc.dram_tensor("cs2", [P, T], f32, kind="ExternalInput")  # [cos;cos]
    sn1 = nc.dram_tensor("sn1", [HALF, T], f32, kind="ExternalInput")  # sin
    # tri[s, u] = 1 iff s <= u (valid upper triangle in the S^T layout)
    tri = nc.dram_tensor("tri", [P, P], bf16, kind="ExternalInput")
    y = nc.dram_tensor("y", [B, HPC * D, C], f32, kind="ExternalOutput")

    with tile.TileContext(nc) as tc:
        with tc.tile_pool(name="const", bufs=1) as cp_, \
             tc.tile_pool(name="qkv", bufs=1) as qp, \
             tc.tile_pool(name="ot", bufs=1) as op_, \
             tc.tile_pool(name="wo", bufs=4) as wop, \
             tc.tile_pool(name="small", bufs=3) as sp, \
             tc.tile_pool(name="pt", bufs=6) as ptp, \
             tc.tile_pool(name="psBsc", bufs=2, space="PSUM") as pssc, \
             tc.tile_pool(name="psBo", bufs=1, space="PSUM") as pso, \
             tc.tile_pool(name="psBsum", bufs=1, space="PSUM") as pssum:

            wq_sb = cp_.tile([P, HPC, KO, D], bf16, tag="wq")
            wk_sb = cp_.tile([P, HPC, KO, D], bf16, tag="wk")
            wv_sb = cp_.tile([P, KO, HPC * D], bf16, tag="wv")
            # first q matmuls need only wq[:, 0]; split the DMA so they
            # start sooner.  cs/sn head chunks come first: RoPE(ti=0) gates
            # the psum-accumulator recycling.
            nc.sync.dma_start(wq_sb[:, 0], wq.ap()[:, 0])
            cs_sb = cp_.tile([P, T], f32, tag="cs")
            sn_sb = cp_.tile([HALF, T], f32, tag="sn")
            tri_sb = cp_.tile([P, P], bf16, tag="tri")
            # ones *matrix* stationary for the denominator matmuls: same
            # cycle cost as a ones-vector (cost ~ moving size), but keeps
            # the PE tile config at (128,128) -- a (128,32) sum config
            # between the score/o matmuls breaks ldweights pipelining --
            # and broadcasts the sums to all partitions, which makes the
            # normalization a plain elementwise multiply (no K=1 broadcast
            # matmul needed).
            ones_f32 = cp_.tile([P, P], f32, tag="ones_f32")
            nc.vector.memset(ones_f32[:], 1.0)
            ones_mat = cp_.tile([P, P], bf16, tag="ones_mat")
            nc.vector.tensor_copy(ones_mat[:], ones_f32[:])

            # PE warmup: the clock ramp (0.65->2.4GHz) needs sustained
            # activity; burn cheap dummy matmuls while the first DMAs land.
            warm_sb = cp_.tile([P, TQ], bf16, tag="warm_sb")
            nc.vector.memset(warm_sb[:], 0.0)
            ps_warm = pssc.tile([P, TA], f32, tag="sc", name="warm")
            for wi in range(24):
                nc.tensor.matmul(ps_warm[:, 0:TQ], ones_mat[:], warm_sb[:],
                                 start=True, stop=True)

            # persistent attention outputs O^T per (b, local head): [d, t]
            oT = [[op_.tile([P, T], bf16, tag=f"oT{b}{hh}", name=f"oT{b}{hh}")
                   for hh in range(HPC)] for b in range(B)]
            qT = [[qp.tile([P, T], bf16, tag=f"qT{b}{hh}", name=f"qT{b}{hh}")
                   for hh in range(HPC)] for b in range(B)]
            kT = [[qp.tile([P, T], bf16, tag=f"kT{b}{hh}", name=f"kT{b}{hh}")
                   for hh in range(HPC)] for b in range(B)]
            vt = [[qp.tile([P, T // P, D], bf16, tag=f"v{b}{hh}",
                           name=f"v{b}{hh}")
                   for hh in range(HPC)] for b in range(B)]

            def gen_qkv(b, xp, psa, rp, state):
                for ti in range(NT):
                    sl = slice(ti * TQ, (ti + 1) * TQ)
                    first_tile = (b == 0 and ti == 0)
                    if state.get("next") is not None:
                        xt = state.pop("next")
                    else:
                        xt = xp.tile([P, KO, TQ], bf16, tag="xt",
                                     name=f"xt{b}_{ti}")
                        if not first_tile:
                            nc.sync.dma_start(xt[:], xTt.ap()[b, ti])
                    cs = cs_sb[:, sl]
                    sn = sn_sb[:, sl]  # [64, TQ] base partition 0

                    def qkmm(w_sb, hh, split_dma=False, csn_after=None):
                        ps = psa.tile([P, TQ], f32, tag="acc",
                                      name=f"acc{b}_{ti}_{hh}")
                        for ko in range(KO):
                            if split_dma and ko % 4 == 0:
                                kos = slice(ko, ko + 4)
                                nc.sync.dma_start(xt[:, kos],
                                                  xTt.ap()[b, ti, :, kos])
                                if ko == 4 and csn_after:
                                    nc.sync.dma_start(cs_sb[:, 0:TQ],
                                                      cs2.ap()[:, 0:TQ])
                                    nc.sync.dma_start(sn_sb[:, 0:TQ],
                                                      sn1.ap()[:, 0:TQ])
                            nc.tensor.matmul(ps[:], w_sb[:, hh, ko, :],
                                             xt[:, ko, :],
                                             start=(ko == 0),
                                             stop=(ko == KO - 1))
                        return ps

                    def rope(ps, dst):
                        # tcos = ps * [cos;cos] (one full mult); tsw
                        # pre-swaps halves: tsw[0:64]=q2*sin,
                        # tsw[64:128]=q1*sin so the gpsimd add/sub reads
                        # align on base partitions.
                        tcos = rp.tile([P, TQ], f32, tag="tcos")
                        tsw = rp.tile([P, TQ], f32, tag="tsw")
                        nc.vector.tensor_mul(tcos[:], ps[:], cs)
                        nc.vector.tensor_mul(tsw[0:HALF, :],
                                             ps[HALF:P, :], sn)
                        nc.vector.tensor_mul(tsw[HALF:P, :],
                                             ps[0:HALF, :], sn)
                        nc.gpsimd.tensor_sub(dst[0:HALF, sl],
                                             tcos[0:HALF, :],
                                             tsw[0:HALF, :])
                        nc.gpsimd.tensor_add(dst[HALF:P, sl],
                                             tcos[HALF:P, :],
                                             tsw[HALF:P, :])

                    if first_tile:
                        # q accums first (need only wq + xt0, DMA'd in ko
                        # quarters); stagger the remaining const DMAs
                        # behind them.
                        ps0 = qkmm(wq_sb, 0, split_dma=True,
                                   csn_after=1)
                        nc.sync.dma_start(wq_sb[:, 1], wq.ap()[:, 1])
                        nc.sync.dma_start(wk_sb[:], wk.ap())
                        yield 1700.0
                        ps1 = qkmm(wq_sb, 1)
                        nc.sync.dma_start(wv_sb[:], wv.ap())
                        rope(ps0, qT[b][0])
                        yield 1700.0
                        psk = qkmm(wk_sb, 0)
                        rope(ps1, qT[b][1])
                        rope(psk, kT[b][0])
                        yield 1700.0
                        psk = qkmm(wk_sb, 1)
                        rope(psk, kT[b][1])
                        yield 1700.0
                    else:
                        for w_sb, dsts in ((wq_sb, qT[b]), (wk_sb, kT[b])):
                            for hh in range(HPC):
                                rope(qkmm(w_sb, hh), dsts[hh])
                                yield 1700.0
                    if b == 0 and ti == 2:
                        nc.sync.dma_start(cs_sb[:, 3 * TQ:],
                                          cs2.ap()[:, 3 * TQ:])
                        nc.sync.dma_start(sn_sb[:, 3 * TQ:],
                                          sn1.ap()[:, 3 * TQ:])
                        nc.sync.dma_start(tri_sb[:], tri.ap())
                    # prefetch next x tile before the v-section so its DMA
                    # gets ahead of lower-priority queue entries
                    nb, nti = (b, ti + 1) if ti + 1 < NT else (b + 1, 0)
                    if nb < B and (nb, nti) > state.get("pref", (-1, -1)):
                        xtn = xp.tile([P, KO, TQ], bf16, tag="xt",
                                      name=f"xt{nb}_{nti}")
                        nc.sync.dma_start(xtn[:], xTt.ap()[nb, nti])
                        state["next"] = xtn
                        state["pref"] = (nb, nti)
                    if b == 0 and ti == 0:
                        nc.sync.dma_start(cs_sb[:, TQ:3 * TQ],
                                          cs2.ap()[:, TQ:3 * TQ])
                        nc.sync.dma_start(sn_sb[:, TQ:3 * TQ],
                                          sn1.ap()[:, TQ:3 * TQ])
                    for sub in range(TQ // P):
                        psv = psa.tile([P, HPC * D], f32, tag="acc",
                                       name=f"accv{b}_{ti}_{sub}")
                        for ko in range(KO):
                            nc.tensor.matmul(
                                psv[:], xt[:, ko, sub * P:(sub + 1) * P],
                                wv_sb[:, ko, :],
                                start=(ko == 0), stop=(ko == KO - 1))
                        tci = ti * (TQ // P) + sub
                        for hh in range(HPC):
                            nc.vector.tensor_copy(
                                vt[b][hh][:, tci, :],
                                psv[:, hh * D:(hh + 1) * D])
                        yield 1700.0

            def gen_attn(b):
                # Both heads interleaved (probs packed in one [P,2,TA] tile
                # so their denominator matmuls merge); o/sum matmuls trail
                # score/exp by one iteration so the PE isn't chained to the
                # Exp latency.  Mask/normalize run on gpsimd -- the vector
                # queue is busy with RoPE and its latency would stall the PE.
                for ta in range(NTA):
                    ps_o = [pso.tile([P, TA], f32, tag=f"o{hh}",
                                     name=f"o{b}_{ta}_{hh}")
                            for hh in range(HPC)]
                    ps_sum = pssum.tile([P, HPC, TA], f32, tag="sum",
                                        name=f"sum{b}_{ta}")
                    smax = (ta + 1) * SCPT - 1
                    pending = []

                    def drain(n, _p=pending, _o=ps_o, _s=ps_sum, _m=smax):
                        while len(_p) > n:
                            s, w, pt2 = _p.pop(0)
                            first, last = (s == 0), (s == _m)
                            for hh in range(HPC):
                                nc.tensor.matmul(_o[hh][:, w],
                                                 vt[b][hh][:, s, :],
                                                 pt2[:, hh, w],
                                                 start=first, stop=last)
                                nc.tensor.matmul(_s[:, hh, w],
                                                 ones_mat[:], pt2[:, hh, w],
                                                 start=first, stop=last)

                    for s in range(smax + 1):
                        j = s - ta * SCPT  # >=0 on the diagonal
                        w0 = P * max(j, 0)
                        w = slice(w0, TA)
                        qsl = slice(ta * TA + w0, (ta + 1) * TA)
                        pt2 = ptp.tile([P, HPC, TA], bf16, tag="pt",
                                       name=f"pt{b}_{ta}_{s}")
                        for hh in range(HPC):
                            ps_sc = pssc.tile([P, TA], f32, tag="sc",
                                              name=f"sc{b}_{ta}_{s}_{hh}")
                            nc.tensor.matmul(
                                ps_sc[:, w], kT[b][hh][:, s * P:(s + 1) * P],
                                qT[b][hh][:, qsl], start=True, stop=True)
                            nc.scalar.activation(
                                pt2[:, hh, w], ps_sc[:, w],
                                mybir.ActivationFunctionType.Exp,
                                scale=SCALE)
                            if j >= 0:  # mask the 128x128 triangle
                                nc.gpsimd.tensor_mul(
                                    pt2[:, hh, w0:w0 + P],
                                    pt2[:, hh, w0:w0 + P], tri_sb[:])
                        pending.append((s, w, pt2))
                        drain(2)
                        yield 5 * (TA - w0) * 0.42
                    drain(0)
                    for hh in range(HPC):
                        recf = sp.tile([P, TA], f32, tag="recf")
                        nc.vector.reciprocal_approx_fast(
                            recf[:], ps_sum[:, hh, :])
                        o_sb = sp.tile([P, TA], f32, tag="o_sb")
                        nc.scalar.copy(o_sb[:], ps_o[hh][:])
                        # write oT pre-shuffled for the out-projection:
                        # oT[p, j*128+u] = O^T[p, t=u*16+j]
                        oview = oT[b][hh].rearrange(
                            "p (j u) -> p u j", j=KO)[
                            :, (TA // 16) * ta:(TA // 16) * (ta + 1), :]
                        nc.gpsimd.tensor_mul(
                            oview,
                            o_sb[:].rearrange("p (u j) -> p u j", j=KO),
                            recf[:].rearrange("p (u j) -> p u j", j=KO))
                    yield 500.0

            def wload(b, cpi):
                wcp = wop.tile([P, KO, TC_], bf16, tag="w",
                               name=f"w{b}_{cpi}")
                nc.sync.dma_start(wcp[:], woutp.ap()[cpi])
                state_w[(b, cpi)] = wcp

            def gen_out(b, psc, cpis=None):
                for cpi in (cpis if cpis is not None else range(NCP)):
                    csl = slice(cpi * TC_, (cpi + 1) * TC_)
                    if (b, cpi) not in state_w:
                        wload(b, cpi)
                    wcp = state_w[(b, cpi)]
                    if b == 0 and cpi == NCP - 1:
                        # prefetch the b=1 pass's first tiles; their slot
                        # WARs resolve as this pass's groups retire.
                        for nc_ in range(4):
                            wload(1, nc_)
                    for hh in range(HPC):
                        psy = psc.tile([P, TC_], f32, tag="y",
                                       name=f"y{b}_{cpi}_{hh}")
                        for j in range(KO):
                            nc.tensor.matmul(psy[:],
                                             oT[b][hh][:, j * P:(j + 1) * P],
                                             wcp[:, j, :],
                                             start=(j == 0),
                                             stop=(j == KO - 1))
                        ysb = sp.tile([P, TC_], f32, tag="ysb")
                        if b == 1:
                            nc.scalar.copy(ysb[:], psy[:])
                        else:
                            nc.vector.tensor_copy(ysb[:], psy[:])
                        nc.sync.dma_start(
                            y.ap()[b, hh * D:(hh + 1) * D, csl], ysb[:])
                        yield 1700.0

            state_w = {}
            qstate = {}
            with tc.tile_pool(name="xt", bufs=2) as xp, \
                 tc.tile_pool(name="rope", bufs=3) as rp, \
                 tc.tile_pool(name="psA", bufs=2, space="PSUM") as psa:
                # phase A: qkv(b=0) alone
                for _ in gen_qkv(0, xp, psa, rp, qstate):
                    pass
                # phase B: attention(b=0) interleaved with qkv(b=1)
                _merge(gen_attn(0), gen_qkv(1, xp, psa, rp, qstate))
                wload(0, 0)
                wload(0, 1)
            with tc.tile_pool(name="psC", bufs=2, space="PSUM") as psc:
                # phase C: attention(b=1) interleaved with out-proj(b=0);
                # the last two b=0 column tiles are held back to bridge the
                # C->D transition while oT[b=1] finishes normalizing.
                _merge(gen_attn(1), gen_out(0, psc, range(NCP - 2)))
                for _ in gen_out(0, psc, range(NCP - 2, NCP)):
                    pass
                # phase D: out-proj(b=1)
                for _ in gen_out(1, psc):
                    pass

    nc.compile()
    return nc


_NC = None


def _get_nc():
    global _NC
    if _NC is None:
        _NC = _build()
    return _NC


def _host_tables():
    pos = np.arange(T, dtype=np.float32)[:, None]
    div = np.exp(np.arange(0, 2 * HALF, 2, dtype=np.float32)
                 * np.float32(-math.log(ROPE_BASE) / (2 * HALF)))
    ang = pos * div[None, :]
    cosv = np.cos(ang).astype(np.float32)   # [T, HALF]
    sinv = np.sin(ang).astype(np.float32)
    cosT = np.ascontiguousarray(cosv.T)     # [HALF, T]
    sinT = np.ascontiguousarray(sinv.T)
    cs2 = np.ascontiguousarray(np.concatenate([cosT, cosT], axis=0))  # [P, T]
    sn1 = sinT
    # triangle mask tri[s, u] = 1 iff s <= u
    uu = np.arange(P)[None, :]
    ss = np.arange(P)[:, None]
    trim = (ss <= uu).astype(ml_dtypes.bfloat16)
    return cs2, sn1, trim


def _make_in_maps(x, Wqkv, Wout):
    x = np.asarray(x, dtype=np.float32)
    Wqkv = np.asarray(Wqkv, dtype=np.float32)
    Wout = np.asarray(Wout, dtype=np.float32)
    assert x.shape == (B, T, C) and Wqkv.shape == (C, 3 * C) \
        and Wout.shape == (C, C)

    cs2, sn1, trim = _host_tables()
    # xTt[b, ti, p, ko, u] = x[b, ti*TQ+u, ko*128+p]
    xTt = np.ascontiguousarray(
        x.astype(ml_dtypes.bfloat16)
        .reshape(B, NT, TQ, KO, P).transpose(0, 1, 4, 3, 2))
    # woutp[cpi, p, j, m] = Wout[j*128+p, cpi*TC_+m]
    woutp = np.ascontiguousarray(
        Wout.astype(ml_dtypes.bfloat16).reshape(KO, P, NCP, TC_)
        .transpose(2, 1, 0, 3))

    in_maps = []
    for core in range(NCORES):
        h0 = core * HPC
        cols = slice(h0 * D, (h0 + HPC) * D)
        ws = []
        for part in range(3):
            w = Wqkv[:, part * C:(part + 1) * C][:, cols] \
                .astype(ml_dtypes.bfloat16)  # [C, HPC*D]
            if part < 2:  # wq/wk: [P, HPC, KO, D]
                ws.append(np.ascontiguousarray(
                    w.reshape(KO, P, HPC, D).transpose(1, 2, 0, 3)))
            else:         # wv: [P, KO, HPC*D]
                ws.append(np.ascontiguousarray(
                    w.reshape(KO, P, HPC * D).transpose(1, 0, 2)))
        in_maps.append({
            "xTt": xTt,
            "wq": ws[0], "wk": ws[1], "wv": ws[2],
            "woutp": woutp,
            "cs2": cs2, "sn1": sn1, "tri": trim,
        })
    return in_maps


def _run(x, Wqkv, Wout, trace=False):
    nc = _get_nc()
    in_maps = _make_in_maps(x, Wqkv, Wout)
    res = run_bass_kernel_spmd(nc, in_maps, core_ids=list(range(NCORES)),
                               trace=trace)
    out = np.empty((B, T, C), dtype=np.float32)
    for core in range(NCORES):
        out[:, core * HPC * D:(core + 1) * HPC * D, :] = \
            res.results[core]["y"]
    return out, res


def kernel(x, Wqkv, Wout):
    out, _ = _run(x, Wqkv, Wout)
    return out
